# revision 1
# baseline (speedup 1.0000x reference)
"""Per-edge dot-product (GNN DotProductPredictor) Bass kernel for 8 trn2 cores.

score[e] = sum_k h[src[e], k] * h[dst[e], k]

Strategy (data-parallel over edges):
  - Edges are sharded contiguously across the 8 cores (400k edges each).
  - h (100k x 64 f32, 25.6MB) stays replicated in each core's HBM.
  - Gathers use the gpsimd `dma_gather` custom instruction (int16 indices,
    256B rows). int16 limits one gather to a 32768-row window, so h is
    viewed as 4 banks and each core's edges are bucketed on the host by
    (src_bank, dst_bank) into 16 buckets. The device executes a static
    chunk plan (fixed #chunks per bucket); unused chunk slots are padded
    with index 0 (a valid row; repeated-row gathers are HBM-cheap).
  - Per chunk of 8192 edges: one idx-tile DMA ([128, 1024] int16, src idx
    in cols [0:512), dst in [512:1024)), two dma_gathers -> [128, 64, 64]
    f32 tiles, DVE multiply (in place), DVE reduce over features, score
    store. The host maps device (bucketed) score positions back to the
    original edge order with one fancy-index per core.
  - Bucket overflow beyond the static capacity (never happens for uniform
    random inputs; capacity is mean + >30 sigma) falls back to numpy on
    the host for the overflowed edges only.
"""

import numpy as np

N_NODES = 100000
N_EDGES = 3200000
D = 64
N_CORES = 8
P = 128

EPC = N_EDGES // N_CORES  # 400000 edges per core

BANK = 32768
N_BANKS = 4  # ceil(100000 / 32768); bank 3 holds rows 98304..99999

CHUNK = 8192
T = CHUNK // P  # 64 score columns per partition
IDXC = CHUNK // 16  # 512 idx columns per gather


def _plan():
    """Static (src_bank, dst_bank, n_chunks) schedule, identical on all cores."""
    plan = []
    for bs in range(N_BANKS):
        for bd in range(N_BANKS):
            big_s = bs < 3
            big_d = bd < 3
            # uniform-random expectation per core: big-big ~42.9k edges,
            # big-small ~2.2k, small-small ~115
            nb = 6 if (big_s and big_d) else 1
            plan.append((bs, bd, nb))
    return plan


PLAN = _plan()
N_CHUNKS_TOTAL = sum(nb for _, _, nb in PLAN)  # 61
CAP = N_CHUNKS_TOTAL * CHUNK  # 499712 device positions per core

_NC = None


def _build_nc(bufs=3, reps=1):
    import contextlib

    import concourse.bacc as bacc
    import concourse.tile as tile
    from concourse import mybir

    nc = bacc.Bacc("TRN2", target_bir_lowering=False)
    h_t = nc.dram_tensor("h", [N_NODES, D], mybir.dt.float32, kind="ExternalInput")
    idx_t = nc.dram_tensor(
        "idx", [N_CHUNKS_TOTAL * P * 2 * IDXC], mybir.dt.int16, kind="ExternalInput"
    )
    out_t = nc.dram_tensor("out", [CAP], mybir.dt.float32, kind="ExternalOutput")

    with tile.TileContext(nc) as tc:
        # reps>1 wraps the whole pipeline in a device-side loop; used only by
        # bench.py to measure per-iteration time above the dispatch floor.
        loop = tc.For_i(0, reps, 1) if reps > 1 else contextlib.nullcontext()
        with loop, tc.tile_pool(name="pool", bufs=bufs) as pool:
            g = 0
            for bs, bd, nb in PLAN:
                src_bank = h_t[bs * BANK : min(N_NODES, (bs + 1) * BANK), :]
                dst_bank = h_t[bd * BANK : min(N_NODES, (bd + 1) * BANK), :]
                for _ in range(nb):
                    ilo = g * P * 2 * IDXC
                    olo = g * CHUNK
                    idxt = pool.tile([P, 2 * IDXC], mybir.dt.int16, tag="idxt")
                    nc.sync.dma_start(
                        out=idxt[:],
                        in_=idx_t[ilo : ilo + P * 2 * IDXC].rearrange(
                            "(p c) -> p c", p=P
                        ),
                    )
                    gs = pool.tile([P, T, D], mybir.dt.float32, tag="gs")
                    gd = pool.tile([P, T, D], mybir.dt.float32, tag="gd")
                    nc.gpsimd.dma_gather(
                        gs[:], src_bank, idxt[:, :IDXC], CHUNK, CHUNK, D,
                        single_packet=False,
                    )
                    nc.gpsimd.dma_gather(
                        gd[:], dst_bank, idxt[:, IDXC:], CHUNK, CHUNK, D,
                        single_packet=False,
                    )
                    nc.vector.tensor_tensor(
                        out=gs[:], in0=gs[:], in1=gd[:], op=mybir.AluOpType.mult
                    )
                    score = pool.tile([P, T], mybir.dt.float32, tag="score")
                    nc.vector.tensor_reduce(
                        out=score[:],
                        in_=gs[:],
                        axis=mybir.AxisListType.X,
                        op=mybir.AluOpType.add,
                    )
                    nc.sync.dma_start(
                        out=out_t[olo : olo + CHUNK].rearrange("(p t) -> p t", p=P),
                        in_=score[:],
                    )
                    g += 1
    nc.compile()
    return nc


def _prep_core(src_c, dst_c, pad=0):
    """Bucket one core's edges; build the device idx array and the mapping
    from device score positions back to edge order.

    pad: filler index for unused chunk slots. Must be a valid row (0):
    the SWDGE ring-space reservation uses the raw num_idxs register, so
    trailing -1 trimming in the gather ucode desyncs the ring and wedges
    the device unless num_idxs_reg is the exact valid count. Padded slots
    re-gather row `pad` of the bank, which is row-buffer friendly.

    Returns (idx_dev [N_CHUNKS_TOTAL*P*2*IDXC] int16,
             edge_pos [EPC] int64 device position per edge (-1 = overflow),
             overflow_mask [EPC] bool)
    """
    bank_s = (src_c >> 15).astype(np.int64)
    bank_d = (dst_c >> 15).astype(np.int64)
    bucket = bank_s * N_BANKS + bank_d
    order = np.argsort(bucket, kind="stable")
    counts = np.bincount(bucket, minlength=16)

    # device chunk layout offsets per bucket
    chunk_base = np.zeros(16, np.int64)
    cap_b = np.zeros(16, np.int64)
    g = 0
    for bi, (bs, bd, nb) in enumerate(PLAN):
        chunk_base[bi] = g * CHUNK
        cap_b[bi] = nb * CHUNK
        g += nb

    # idx lists in device (bucketed) order, padded with -1 to CAP
    s_loc = np.full(CAP, pad, np.int16)
    d_loc = np.full(CAP, pad, np.int16)
    # device position q within bucket bi -> global slot chunk_base[bi] + q
    edge_pos = np.full(EPC, -1, np.int64)
    overflow = np.zeros(EPC, bool)

    start = 0
    for bi in range(16):
        m = int(counts[bi])
        take = min(m, int(cap_b[bi]))
        e = order[start : start + take]
        slots = chunk_base[bi] + np.arange(take)
        s_loc[slots] = (src_c[e] & (BANK - 1)).astype(np.int16)
        d_loc[slots] = (dst_c[e] & (BANK - 1)).astype(np.int16)
        # score of slot q lands at device out position:
        #   chunk g = q // CHUNK, i = q % CHUNK, p = i % P, c = i // P
        #   out pos = g*CHUNK + p*T + c
        i = slots % CHUNK
        edge_pos[e] = (slots // CHUNK) * CHUNK + (i % P) * T + (i // P)
        if m > take:
            overflow[order[start + take : start + m]] = True
        start += m

    # idx tile layout per chunk: [P, 2*IDXC] int16; gather position i ->
    # column i//16, partition-row i%16, replicated across the 8 groups
    def tiles(loc):
        a = loc.reshape(N_CHUNKS_TOTAL, IDXC, 16).transpose(0, 2, 1)  # [g,16,IDXC]
        return np.broadcast_to(a[:, None, :, :], (N_CHUNKS_TOTAL, 8, 16, IDXC))

    idx_dev = np.empty((N_CHUNKS_TOTAL, 8, 16, 2 * IDXC), np.int16)
    idx_dev[:, :, :, :IDXC] = tiles(s_loc)
    idx_dev[:, :, :, IDXC:] = tiles(d_loc)
    return idx_dev.reshape(-1), edge_pos, overflow


def kernel(h, src, dst, _trace=False):
    global _NC
    from concourse import bass_utils

    h = np.ascontiguousarray(np.asarray(h), dtype=np.float32)
    src = np.asarray(src).astype(np.int32)
    dst = np.asarray(dst).astype(np.int32)

    if _NC is None:
        _NC = _build_nc()

    in_maps = []
    maps = []
    for c in range(N_CORES):
        lo = c * EPC
        idx_dev, edge_pos, overflow = _prep_core(src[lo : lo + EPC], dst[lo : lo + EPC])
        in_maps.append({"h": h, "idx": idx_dev})
        maps.append((edge_pos, overflow))

    res = bass_utils.run_bass_kernel_spmd(
        _NC, in_maps, core_ids=list(range(N_CORES)), trace=_trace
    )

    out = np.empty(N_EDGES, np.float32)
    for c in range(N_CORES):
        lo = c * EPC
        edge_pos, overflow = maps[c]
        dev_out = res.results[c]["out"]
        ok = ~overflow
        out[lo : lo + EPC][ok] = dev_out[edge_pos[ok]]
        if overflow.any():  # static capacity exceeded: host fallback
            e = np.nonzero(overflow)[0]
            s = src[lo : lo + EPC][e].astype(np.int64)
            d_ = dst[lo : lo + EPC][e].astype(np.int64)
            out[lo : lo + EPC][e] = np.einsum("ij,ij->i", h[s], h[d_])
    out = out.reshape(N_EDGES, 1)
    if _trace:
        return out, res
    return out



# revision 3
# speedup vs baseline: 2.4234x; 2.4234x over previous
"""Per-edge dot-product (GNN DotProductPredictor) Bass kernel for 8 trn2 cores.

score[e] = sum_k h[src[e], k] * h[dst[e], k]

Strategy (data-parallel over edges):
  - Edges are sharded contiguously across the 8 cores (400k edges each).
  - h (100k x 64 f32, 25.6MB) stays replicated in each core's HBM.
  - Gathers use the gpsimd `dma_gather` custom instruction (int16 indices,
    256B rows). int16 limits one gather to a 32768-row window, so h is
    viewed as 4 banks and each core's edges are bucketed on the host by
    (src_bank, dst_bank) into 16 buckets. The device executes a static
    chunk plan (fixed #chunks per bucket); unused chunk slots are padded
    with index 0 (a valid row; repeated-row gathers are HBM-cheap).
  - Per chunk of 8192 edges: one idx-tile DMA ([128, 1024] int16, src idx
    in cols [0:512), dst in [512:1024)), two dma_gathers -> [128, 64, 64]
    f32 tiles, DVE multiply (in place), DVE reduce over features, score
    store. The host maps device (bucketed) score positions back to the
    original edge order with one fancy-index per core.
  - Bucket overflow beyond the static capacity (never happens for uniform
    random inputs; capacity is mean + >30 sigma) falls back to numpy on
    the host for the overflowed edges only.
"""

import numpy as np

N_NODES = 100000
N_EDGES = 3200000
D = 64
N_CORES = 8
P = 128

EPC = N_EDGES // N_CORES  # 400000 edges per core

BANK = 32768
N_BANKS = 4  # ceil(100000 / 32768); bank 3 holds rows 98304..99999

CHUNK = 8192
T = CHUNK // P  # 64 score columns per partition
IDXC = CHUNK // 16  # 512 idx columns per gather


def _plan():
    """Static (src_bank, dst_bank, n_chunks) schedule, identical on all cores."""
    plan = []
    for bs in range(N_BANKS):
        for bd in range(N_BANKS):
            big_s = bs < 3
            big_d = bd < 3
            # uniform-random expectation per core: big-big ~42.9k edges,
            # big-small ~2.2k, small-small ~115
            nb = 6 if (big_s and big_d) else 1
            plan.append((bs, bd, nb))
    return plan


PLAN = _plan()
N_CHUNKS_TOTAL = sum(nb for _, _, nb in PLAN)  # 61
CAP = N_CHUNKS_TOTAL * CHUNK  # 499712 device positions per core

_NC = None


def _build_nc(bufs=3, reps=1):
    import contextlib

    import concourse.bacc as bacc
    import concourse.tile as tile
    from concourse import mybir

    nc = bacc.Bacc("TRN2", target_bir_lowering=False, num_swdge_queues=4)
    h_t = nc.dram_tensor("h", [N_NODES, D], mybir.dt.float32, kind="ExternalInput")
    idx_t = nc.dram_tensor(
        "idx", [N_CHUNKS_TOTAL * P * 2 * IDXC], mybir.dt.int16, kind="ExternalInput"
    )
    out_t = nc.dram_tensor("out", [CAP], mybir.dt.float32, kind="ExternalOutput")

    with tile.TileContext(nc) as tc:
        # reps>1 wraps the whole pipeline in a device-side loop; used only by
        # bench.py to measure per-iteration time above the dispatch floor.
        loop = tc.For_i(0, reps, 1) if reps > 1 else contextlib.nullcontext()
        with loop, tc.tile_pool(name="pool", bufs=bufs) as pool:
            g = 0
            for bs, bd, nb in PLAN:
                src_bank = h_t[bs * BANK : min(N_NODES, (bs + 1) * BANK), :]
                dst_bank = h_t[bd * BANK : min(N_NODES, (bd + 1) * BANK), :]
                for _ in range(nb):
                    ilo = g * P * 2 * IDXC
                    olo = g * CHUNK
                    idxt = pool.tile([P, 2 * IDXC], mybir.dt.int16, tag="idxt")
                    nc.sync.dma_start(
                        out=idxt[:],
                        in_=idx_t[ilo : ilo + P * 2 * IDXC].rearrange(
                            "(p c) -> p c", p=P
                        ),
                    )
                    gs = pool.tile([P, T, D], mybir.dt.float32, tag="gs")
                    gd = pool.tile([P, T, D], mybir.dt.float32, tag="gd")
                    nc.gpsimd.dma_gather(
                        gs[:], src_bank, idxt[:, :IDXC], CHUNK, CHUNK, D,
                        single_packet=False, queue_num=(2 * g) % 4,
                    )
                    nc.gpsimd.dma_gather(
                        gd[:], dst_bank, idxt[:, IDXC:], CHUNK, CHUNK, D,
                        single_packet=False, queue_num=(2 * g + 1) % 4,
                    )
                    nc.vector.tensor_tensor(
                        out=gs[:], in0=gs[:], in1=gd[:], op=mybir.AluOpType.mult
                    )
                    score = pool.tile([P, T], mybir.dt.float32, tag="score")
                    nc.vector.tensor_reduce(
                        out=score[:],
                        in_=gs[:],
                        axis=mybir.AxisListType.X,
                        op=mybir.AluOpType.add,
                    )
                    nc.sync.dma_start(
                        out=out_t[olo : olo + CHUNK].rearrange("(p t) -> p t", p=P),
                        in_=score[:],
                    )
                    g += 1
    nc.compile()
    return nc


def _prep_core(src_c, dst_c, pad=0):
    """Bucket one core's edges; build the device idx array and the mapping
    from device score positions back to edge order.

    pad: filler index for unused chunk slots. Must be a valid row (0):
    the SWDGE ring-space reservation uses the raw num_idxs register, so
    trailing -1 trimming in the gather ucode desyncs the ring and wedges
    the device unless num_idxs_reg is the exact valid count. Padded slots
    re-gather row `pad` of the bank, which is row-buffer friendly.

    Returns (idx_dev [N_CHUNKS_TOTAL*P*2*IDXC] int16,
             edge_pos [EPC] int64 device position per edge (-1 = overflow),
             overflow_mask [EPC] bool)
    """
    bank_s = (src_c >> 15).astype(np.int64)
    bank_d = (dst_c >> 15).astype(np.int64)
    bucket = bank_s * N_BANKS + bank_d
    order = np.argsort(bucket, kind="stable")
    counts = np.bincount(bucket, minlength=16)

    # device chunk layout offsets per bucket
    chunk_base = np.zeros(16, np.int64)
    cap_b = np.zeros(16, np.int64)
    g = 0
    for bi, (bs, bd, nb) in enumerate(PLAN):
        chunk_base[bi] = g * CHUNK
        cap_b[bi] = nb * CHUNK
        g += nb

    # idx lists in device (bucketed) order, padded with -1 to CAP
    s_loc = np.full(CAP, pad, np.int16)
    d_loc = np.full(CAP, pad, np.int16)
    # device position q within bucket bi -> global slot chunk_base[bi] + q
    edge_pos = np.full(EPC, -1, np.int64)
    overflow = np.zeros(EPC, bool)

    start = 0
    for bi in range(16):
        m = int(counts[bi])
        take = min(m, int(cap_b[bi]))
        e = order[start : start + take]
        slots = chunk_base[bi] + np.arange(take)
        s_loc[slots] = (src_c[e] & (BANK - 1)).astype(np.int16)
        d_loc[slots] = (dst_c[e] & (BANK - 1)).astype(np.int16)
        # score of slot q lands at device out position:
        #   chunk g = q // CHUNK, i = q % CHUNK, p = i % P, c = i // P
        #   out pos = g*CHUNK + p*T + c
        i = slots % CHUNK
        edge_pos[e] = (slots // CHUNK) * CHUNK + (i % P) * T + (i // P)
        if m > take:
            overflow[order[start + take : start + m]] = True
        start += m

    # idx tile layout per chunk: [P, 2*IDXC] int16; gather position i ->
    # column i//16, partition-row i%16, replicated across the 8 groups
    def tiles(loc):
        a = loc.reshape(N_CHUNKS_TOTAL, IDXC, 16).transpose(0, 2, 1)  # [g,16,IDXC]
        return np.broadcast_to(a[:, None, :, :], (N_CHUNKS_TOTAL, 8, 16, IDXC))

    idx_dev = np.empty((N_CHUNKS_TOTAL, 8, 16, 2 * IDXC), np.int16)
    idx_dev[:, :, :, :IDXC] = tiles(s_loc)
    idx_dev[:, :, :, IDXC:] = tiles(d_loc)
    return idx_dev.reshape(-1), edge_pos, overflow


def kernel(h, src, dst, _trace=False):
    global _NC
    from concourse import bass_utils

    h = np.ascontiguousarray(np.asarray(h), dtype=np.float32)
    src = np.asarray(src).astype(np.int32)
    dst = np.asarray(dst).astype(np.int32)

    if _NC is None:
        _NC = _build_nc()

    in_maps = []
    maps = []
    for c in range(N_CORES):
        lo = c * EPC
        idx_dev, edge_pos, overflow = _prep_core(src[lo : lo + EPC], dst[lo : lo + EPC])
        in_maps.append({"h": h, "idx": idx_dev})
        maps.append((edge_pos, overflow))

    res = bass_utils.run_bass_kernel_spmd(
        _NC, in_maps, core_ids=list(range(N_CORES)), trace=_trace
    )

    out = np.empty(N_EDGES, np.float32)
    for c in range(N_CORES):
        lo = c * EPC
        edge_pos, overflow = maps[c]
        dev_out = res.results[c]["out"]
        ok = ~overflow
        out[lo : lo + EPC][ok] = dev_out[edge_pos[ok]]
        if overflow.any():  # static capacity exceeded: host fallback
            e = np.nonzero(overflow)[0]
            s = src[lo : lo + EPC][e].astype(np.int64)
            d_ = dst[lo : lo + EPC][e].astype(np.int64)
            out[lo : lo + EPC][e] = np.einsum("ij,ij->i", h[s], h[d_])
    out = out.reshape(N_EDGES, 1)
    if _trace:
        return out, res
    return out

